# revision 7
# baseline (speedup 1.0000x reference)
# BiLSTM-CRF NLL kernel for 8x Trainium2 NeuronCores (Bass/Tile), v2.
#
# Data-parallel over batch (16 seqs/core). Per core:
#   Host: embedding gather + transpose (eT), weight packing.
#   A0/A1 (per layer): hoisted input-side GEMM over all timesteps:
#       gx[l,d] = actT-chunks @ w_ih[d].T  (weights moving, activations
#       stationary, full 128-wide PE) -> bf16 -> DRAM scratch.
#   R0/R1 (per layer): recurrence, two software-pipelined chains (fwd, bwd).
#       Per dir per step: DMA gx slice [16,1024] -> gxs (row 16 = bias);
#       gates PSUM [48,512] = i17-preload(gx+bias) + hT@W_hh (4 MMs);
#       one ACT tanh(0.5*gps) -> T48; aligned elementwise tail (bf16);
#       PE-transpose h -> hT storage (doubles as next-step stationary).
#       Conventions: C2 = 2c, stored h = 2h (downstream weights pre-halved),
#       g-gate columns pre-doubled (ACT computes tanh(0.5*x)).
#   P3/P4: FC -> emissions em3 = em - 3 (drift fold); expem = exp(em3);
#       gold-path score via one-hot matmuls + strided reduces.
#   P5: CRF partition function in exp space; v_b = logZ_dev - score_dev.
# Host: output = mean over all 128 v_b.
import sys
import numpy as np

sys.path.insert(0, "/opt/trn_rl_repo")

import ml_dtypes
from contextlib import ExitStack

import concourse.bass as bass
import concourse.tile as tile
from concourse import bacc, mybir
from concourse.bass_utils import run_bass_kernel_spmd
from concourse.masks import make_identity

f32 = mybir.dt.float32
bf16 = mybir.dt.bfloat16
i32 = mybir.dt.int32
AF = mybir.ActivationFunctionType
ALU = mybir.AluOpType
bfnp = ml_dtypes.bfloat16

B, L, V, T, E, H = 128, 512, 30000, 20, 256, 256
NC_CORES = 8
BC = B // NC_CORES            # 16 sequences per core


def _pack_gates(Wt, in_scale):
    # Wt: [K, 1024] = w.T ; scale inputs, double g-gate cols (tanh(0.5x) trick)
    M = Wt.astype(np.float64) * in_scale
    M[:, 512:768] *= 2.0
    return M


def _build_host_inputs(x, tags, emb, w_ih0, w_hh0, b_ih0, b_hh0,
                       w_ih1, w_hh1, b_ih1, b_hh1, fc_W, fc_b,
                       crf_trans, crf_start, crf_end, Lsteps):
    ntb = Lsteps * BC
    shared = {}

    # --- x-GEMM moving weights: col ((d*K+k)*2+h)*512 ---
    def pack_wx(w_ih, K, in_scale):
        out = np.zeros((128, 2 * K * 2 * 512), dtype=np.float64)
        for d in range(2):
            M = _pack_gates(w_ih[d].T, in_scale)   # [K*128, 1024]
            for k in range(K):
                for h in range(2):
                    out[:, ((d * K + k) * 2 + h) * 512:((d * K + k) * 2 + h + 1) * 512] = \
                        M[k * 128:(k + 1) * 128, h * 512:(h + 1) * 512]
        return out.astype(bfnp)

    shared["wx0"] = pack_wx(w_ih0, 2, 1.0)
    shared["wx1"] = pack_wx(w_ih1, 4, 0.5)
    shared["wh0"] = pack_wx(w_hh0, 2, 0.5)
    shared["wh1"] = pack_wx(w_hh1, 2, 0.5)

    # --- biases: [4, 1024] rows (l*2+d), g-cols doubled ---
    bias = np.zeros((4, 1024), dtype=np.float64)
    for l, (bi, bh) in enumerate(((b_ih0, b_hh0), (b_ih1, b_hh1))):
        for d in range(2):
            v = (bi[d] + bh[d]).astype(np.float64)
            v[512:768] *= 2.0
            bias[l * 2 + d] = v
    shared["biasv"] = bias.astype(bfnp)

    shared["i17"] = np.vstack([np.eye(BC), np.ones((1, BC))]).astype(bfnp)

    fcp = np.zeros((128, 4 * T), dtype=np.float64)
    fw = fc_W.T * 0.5
    for k in range(4):
        fcp[:, k * T:(k + 1) * T] = fw[k * 128:(k + 1) * 128]
    shared["fcp"] = fcp.astype(bfnp)
    shared["fcb3"] = (fc_b.astype(np.float64) - 3.0)[None, :].astype(np.float32)
    shared["mexp"] = np.exp(crf_trans.astype(np.float64)).astype(np.float32)
    shared["transb"] = crf_trans.astype(bfnp)
    shared["startexp"] = np.exp(crf_start.astype(np.float64)).astype(np.float32)[:, None]
    shared["startT"] = crf_start.astype(np.float32)[:, None]
    shared["endexp"] = np.exp(crf_end.astype(np.float64)).astype(np.float32)[:, None]
    shared["endT"] = crf_end.astype(np.float32)[:, None]
    shared["iota20"] = np.arange(T, dtype=np.float32)[:, None]

    per_core = []
    embf = np.asarray(emb, np.float32)
    for c in range(NC_CORES):
        xc = np.asarray(x)[c * BC:(c + 1) * BC, :Lsteps]
        tc_ = np.asarray(tags)[c * BC:(c + 1) * BC, :Lsteps]
        e = embf[xc]                                # [BC, Ls, E]
        eTf = e.transpose(2, 1, 0).reshape(E, ntb)  # [E, t*BC+b]
        eT = np.concatenate([eTf[0:128], eTf[128:256]], axis=1)  # [128, 2*ntb]
        tgf = tc_.T.reshape(-1)
        per_core.append({"eT": np.ascontiguousarray(eT.astype(bfnp)),
                         "tg": tgf.astype(bfnp)[None, :]})
    return shared, per_core


def _emit_phase_a(nc, pools, Lsteps, chunks, wx, K, gxd, biasref):
    # chunks: list of (tile, col_base) of K stationary sources [128, ntb-slices]
    # gxd: [2] dram handles; writes gx[d][:, c*1024:(c+1)*1024]
    psum_x, work, stage = pools["psum_x"], pools["work"], pools["stage"]
    nchunk = Lsteps * BC // 128
    for c in range(nchunk):
        for d in range(2):
            ps_if = psum_x.tile([128, 512], f32, tag="pe")
            ps_go = psum_x.tile([128, 512], f32, tag="pe")
            for k in range(K):
                et, base = chunks[k]
                st = et[:, base + c * 128: base + (c + 1) * 128]
                wc = ((d * K + k) * 2) * 512
                nc.tensor.matmul(ps_if[:], st, wx[:, wc:wc + 512],
                                 start=(k == 0), stop=(k == K - 1))
                nc.tensor.matmul(ps_go[:], st, wx[:, wc + 512:wc + 1024],
                                 start=(k == 0), stop=(k == K - 1))
            gxc = stage.tile([128, 1024], bf16, tag="gxc")
            nc.scalar.activation(gxc[:, 0:512], ps_if[:], AF.Copy)
            nc.scalar.activation(gxc[:, 512:1024], ps_go[:], AF.Copy)
            nc.sync.dma_start(gxd[d][:, c * 1024:(c + 1) * 1024], gxc[:])


def _emit_recurrence(nc, pools, lyr, Lsteps, wh, gxd, biasv, hfT, hbT,
                     i17, ident16):
    work, state, psum_g, psum_t = (pools["work"], pools["state"],
                                   pools["psum_g"], pools["psum_t"])
    ntb = Lsteps * BC
    gxs = {}
    C2 = {}
    for d in range(2):
        for p in range(2):
            t_ = state.tile([17, 1024], bf16, tag=f"gxs{d}{p}")
            nc.sync.dma_start(t_[16:17, :], biasv[2 * lyr + d:2 * lyr + d + 1, :])
            gxs[(d, p)] = t_
    hT = {0: hfT, 1: hbT}
    gps_live = {}

    def dma_gx(d, s):
        t = s if d == 0 else Lsteps - 1 - s
        g = gxs[(d, s % 2)]
        src = gxd[d][(t % 8) * BC:(t % 8 + 1) * BC,
                     (t // 8) * 1024:(t // 8 + 1) * 1024]
        nc.sync.dma_start(g[0:BC, :], src)

    def mm(d, s):
        t = s if d == 0 else Lsteps - 1 - s
        g = gxs[(d, s % 2)]
        gps = psum_g.tile([48, 512], f32, tag="gps")
        gps_live[d] = gps
        last = (s == 0)
        nc.tensor.matmul(gps[0:16, :], i17[:], g[:, 0:512],
                         start=True, stop=last, tile_position=(0, 0))
        nc.tensor.matmul(gps[32:48, :], i17[:], g[:, 512:1024],
                         start=True, stop=last, tile_position=(0, 32))
        if s > 0:
            t_prev = t - 1 if d == 0 else t + 1
            for k in range(2):
                st = hT[d][:, k * ntb + t_prev * BC: k * ntb + (t_prev + 1) * BC]
                wc = ((d * 2 + k) * 2) * 512
                nc.tensor.matmul(gps[0:16, :], st, wh[:, wc:wc + 512],
                                 start=False, stop=(k == 1), tile_position=(0, 0))
                nc.tensor.matmul(gps[32:48, :], st, wh[:, wc + 512:wc + 1024],
                                 start=False, stop=(k == 1), tile_position=(0, 32))

    def tail(d, s):
        t = s if d == 0 else Lsteps - 1 - s
        gps = gps_live[d]
        T48 = work.tile([48, 512], bf16, tag=f"T48{d}")
        nc.scalar.activation(T48[:], gps[:], AF.Tanh, scale=0.5)
        Tgo = work.tile([16, 512], bf16, tag=f"Tgo{d}")
        nc.vector.tensor_copy(Tgo[:], T48[32:48, :])
        A = work.tile([16, 256], bf16, tag=f"A{d}")
        nc.vector.scalar_tensor_tensor(A[:], T48[0:16, 0:256], 1.0,
                                       Tgo[:, 0:256], op0=ALU.add, op1=ALU.mult)
        if s > 0:
            Bt = work.tile([16, 256], bf16, tag=f"Bt{d}")
            nc.vector.scalar_tensor_tensor(Bt[:], T48[0:16, 256:512], 1.0,
                                           C2[d], op0=ALU.add, op1=ALU.mult)
            C2n = state.tile([16, 256], bf16, tag=f"C2{d}{s % 2}")
            nc.vector.scalar_tensor_tensor(C2n[:], Bt[:], 0.5, A[:],
                                           op0=ALU.mult, op1=ALU.add)
            C2[d] = C2n
        else:
            C2[d] = A
        TC = work.tile([16, 256], bf16, tag=f"TC{d}")
        nc.scalar.activation(TC[:], C2[d][:], AF.Tanh, scale=0.5)
        Hh = work.tile([16, 256], bf16, tag=f"Hh{d}")
        nc.vector.scalar_tensor_tensor(Hh[:], Tgo[:, 256:512], 1.0, TC[:],
                                       op0=ALU.add, op1=ALU.mult)
        tps = psum_t.tile([128, 32], bf16, tag="tps")
        nc.tensor.transpose(tps[:, 0:16], Hh[:, 0:128], ident16[:])
        nc.tensor.transpose(tps[:, 16:32], Hh[:, 128:256], ident16[:])
        dst = hT[d][:].rearrange("p (k n) -> p k n", k=2, n=ntb)[:, :, t * BC:(t + 1) * BC]
        nc.vector.tensor_copy(dst, tps[:].rearrange("p (k c) -> p k c", k=2))

    # software-pipelined emission: two chains (fwd=0, bwd=1)
    dma_gx(0, 0)
    dma_gx(1, 0)
    mm(0, 0)
    mm(1, 0)
    for s in range(1, Lsteps):
        dma_gx(0, s)
        dma_gx(1, s)
        tail(1, s - 1)
        mm(1, s)
        tail(0, s - 1)
        mm(0, s)
    tail(1, Lsteps - 1)
    tail(0, Lsteps - 1)


def build_nc(Lsteps=L, debug_outs=()):
    nc = bacc.Bacc("TRN2", target_bir_lowering=False, debug=False)
    ntb = Lsteps * BC
    nchunk = ntb // 128
    dp = lambda n, s, dt: nc.declare_dram_parameter(n, s, dt, isOutput=False).ap()
    eT_i = dp("eT", [128, 2 * ntb], bf16)
    tg_i = dp("tg", [1, ntb], bf16)
    wx0_i = dp("wx0", [128, 2 * 2 * 2 * 512], bf16)
    wx1_i = dp("wx1", [128, 2 * 4 * 2 * 512], bf16)
    wh0_i = dp("wh0", [128, 2 * 2 * 2 * 512], bf16)
    wh1_i = dp("wh1", [128, 2 * 2 * 2 * 512], bf16)
    biasv_i = dp("biasv", [4, 1024], bf16)
    i17_i = dp("i17", [17, BC], bf16)
    fcp_i = dp("fcp", [128, 4 * T], bf16)
    fcb3_i = dp("fcb3", [1, T], f32)
    mexp_i = dp("mexp", [T, T], f32)
    transb_i = dp("transb", [T, T], bf16)
    startexp_i = dp("startexp", [T, 1], f32)
    startT_i = dp("startT", [T, 1], f32)
    endexp_i = dp("endexp", [T, 1], f32)
    endT_i = dp("endT", [T, 1], f32)
    iota_i = dp("iota20", [T, 1], f32)
    v_o = nc.declare_dram_parameter("v", [1, BC], f32, isOutput=True).ap()
    dbg = {}
    if "h0f" in debug_outs:
        for nm in ("h0f", "h0b", "h1f", "h1b"):
            dbg[nm] = nc.declare_dram_parameter(nm, [128, 2 * ntb], bf16,
                                                isOutput=True).ap()
    if "score" in debug_outs:
        dbg["score"] = nc.declare_dram_parameter("score", [1, BC], f32, isOutput=True).ap()
        dbg["SL"] = nc.declare_dram_parameter("SL", [T, BC], f32, isOutput=True).ap()
        dbg["expem"] = nc.declare_dram_parameter("expem", [T, ntb], bf16, isOutput=True).ap()

    # DRAM scratch for hoisted gates: [l][d] -> [128, nchunk*1024] bf16
    gxd = [[nc.dram_tensor(f"gx{l}{d}", (128, nchunk * 1024), bf16,
                           kind="Internal").ap()
            for d in range(2)] for l in range(2)]

    with tile.TileContext(nc) as tc, ExitStack() as ctx:
        consts = ctx.enter_context(tc.tile_pool(name="consts", bufs=1))
        wpool = ctx.enter_context(tc.tile_pool(name="wpool", bufs=1))
        slotA = ctx.enter_context(tc.tile_pool(name="slotA", bufs=1))
        hbuf = ctx.enter_context(tc.tile_pool(name="hbuf", bufs=1))
        state = ctx.enter_context(tc.tile_pool(name="state", bufs=1))
        work = ctx.enter_context(tc.tile_pool(name="work", bufs=2))
        stage = ctx.enter_context(tc.tile_pool(name="stage", bufs=2))
        psum_x = ctx.enter_context(tc.tile_pool(name="psum_x", bufs=3, space="PSUM"))
        psum_g = ctx.enter_context(tc.tile_pool(name="psum_g", bufs=3, space="PSUM"))
        psum_t = ctx.enter_context(tc.tile_pool(name="psum_t", bufs=2, space="PSUM"))
        pools = dict(work=work, state=state, stage=stage,
                     psum_x=psum_x, psum_g=psum_g, psum_t=psum_t)

        ident = consts.tile([128, 128], bf16)
        make_identity(nc, ident)
        ident16 = ident[0:16, 0:16]
        ones512f = consts.tile([1, 512], f32)
        nc.vector.memset(ones512f[:], 1.0)
        ones20f = consts.tile([T, 1], f32)
        nc.vector.memset(ones20f[:], 1.0)
        ones1_20 = consts.tile([1, T], bf16)
        nc.vector.memset(ones1_20[:], 1.0)

        def cload(name, src, shape, dt):
            t = consts.tile(shape, dt, tag=name)
            nc.sync.dma_start(t[:], src[:])
            return t
        i17 = cload("i17", i17_i, [17, BC], bf16)
        biasv = cload("biasv", biasv_i, [4, 1024], bf16)
        mexp = cload("mexp", mexp_i, [T, T], f32)
        transb = cload("transb", transb_i, [T, T], bf16)
        startexp = cload("startexp", startexp_i, [T, 1], f32)
        startT = cload("startT", startT_i, [T, 1], f32)
        endexp = cload("endexp", endexp_i, [T, 1], f32)
        endT = cload("endT", endT_i, [T, 1], f32)
        iota20 = cload("iota20", iota_i, [T, 1], f32)
        fcb3 = cload("fcb3", fcb3_i, [1, T], f32)
        fcp = cload("fcp", fcp_i, [128, 4 * T], bf16)

        wx = wpool.tile([128, 2 * 4 * 2 * 512], bf16, tag="wx")
        wh = wpool.tile([128, 2 * 2 * 2 * 512], bf16, tag="wh")

        # ---------- layer 0 ----------
        eT = slotA.tile([128, 2 * ntb], bf16, tag="slotA")
        nc.sync.dma_start(eT[:], eT_i[:])
        nc.sync.dma_start(wx[:, 0:4096], wx0_i[:])
        nc.sync.dma_start(wh[:], wh0_i[:])
        _emit_phase_a(nc, pools, Lsteps, [(eT, 0), (eT, ntb)], wx, 2,
                      gxd[0], biasv)
        h0fT = hbuf.tile([128, 2 * ntb], bf16, tag="hb_f")
        h0bT = hbuf.tile([128, 2 * ntb], bf16, tag="hb_b")
        _emit_recurrence(nc, pools, 0, Lsteps, wh, gxd[0], biasv,
                         h0fT, h0bT, i17, ident16)

        # ---------- layer 1 ----------
        nc.sync.dma_start(wx[:], wx1_i[:])
        nc.sync.dma_start(wh[:], wh1_i[:])
        _emit_phase_a(nc, pools, Lsteps,
                      [(h0fT, 0), (h0fT, ntb), (h0bT, 0), (h0bT, ntb)],
                      wx, 4, gxd[1], biasv)
        h1fT = slotA.tile([128, 2 * ntb], bf16, tag="slotA")
        h1bT = hbuf.tile([128, 2 * ntb], bf16, tag="hb_f")
        _emit_recurrence(nc, pools, 1, Lsteps, wh, gxd[1], biasv,
                         h1fT, h1bT, i17, ident16)
        if "h0f" in dbg:
            nc.sync.dma_start(dbg["h0f"][:], h0fT[:])
            nc.sync.dma_start(dbg["h0b"][:], h0bT[:])
            nc.sync.dma_start(dbg["h1f"][:], h1fT[:])
            nc.sync.dma_start(dbg["h1b"][:], h1bT[:])

        # ---------- P3+P4: FC, expem, one-hot, score pieces (chunked) ----------
        psum_e = psum_x
        psum_s = psum_g
        expem = state.tile([T, ntb], bf16, tag="expem")
        pile = state.tile([T, BC], f32, tag="pile")
        nc.vector.memset(pile[:], 0.0)
        red = work.tile([T, BC], f32, tag="red")
        ncol = 512
        nchunks = (ntb + ncol - 1) // ncol
        for ci in range(nchunks):
            n0 = ci * ncol
            nn = min(ncol, ntb - n0)
            em_ps = psum_e.tile([T, ncol], f32, tag="pe")
            nc.tensor.matmul(em_ps[:, 0:nn], fcb3[:], ones512f[:, 0:nn],
                             start=True, stop=False)
            for k in range(4):
                ht = h1fT if k < 2 else h1bT
                kk = k % 2
                nc.tensor.matmul(em_ps[:, 0:nn], fcp[:, k * T:(k + 1) * T],
                                 ht[:, kk * ntb + n0: kk * ntb + n0 + nn],
                                 start=False, stop=(k == 3))
            nc.scalar.activation(expem[:, n0:n0 + nn], em_ps[:, 0:nn], AF.Exp)
            # one-hot of tags for this chunk (+BC-shifted variant for transitions)
            tgc = stage.tile([1, ncol + BC], bf16, tag="tgc")
            nsh = min(nn + BC, ntb - n0)
            nc.sync.dma_start(tgc[:, 0:nsh], tg_i[:, n0:n0 + nsh])
            tg_ps = psum_s.tile([T, ncol], f32, tag="gps")
            nc.tensor.matmul(tg_ps[:, 0:nn], ones1_20[:], tgc[:, 0:nn],
                             start=True, stop=True)
            M1c = work.tile([T, ncol + BC], bf16, tag="M1c")
            nc.vector.tensor_scalar(M1c[:, 0:nn], tg_ps[:, 0:nn], iota20[:], None,
                                    op0=ALU.is_equal)
            if nsh > nn:
                tg_ps2 = psum_s.tile([T, ncol], f32, tag="gps")
                nc.tensor.matmul(tg_ps2[:, 0:nsh - nn], ones1_20[:], tgc[:, nn:nsh],
                                 start=True, stop=True)
                nc.vector.tensor_scalar(M1c[:, nn:nsh], tg_ps2[:, 0:nsh - nn],
                                        iota20[:], None, op0=ALU.is_equal)
            # emission part of score
            EMSc = work.tile([T, ncol], f32, tag="EMSc")
            nc.vector.tensor_mul(EMSc[:, 0:nn], em_ps[:, 0:nn], M1c[:, 0:nn])
            nc.vector.reduce_sum(red[:],
                                 EMSc[:, 0:nn].rearrange("p (t b) -> p b t", b=BC),
                                 axis=mybir.AxisListType.X)
            nc.vector.tensor_add(pile[:], pile[:], red[:])
            # transition part: G[:,tb]*M1[:,tb+BC]
            g_ps = psum_s.tile([T, ncol], f32, tag="gps")
            nc.tensor.matmul(g_ps[:, 0:nn], transb[:], M1c[:, 0:nn],
                             start=True, stop=True)
            n3 = nn if n0 + nn < ntb else nn - BC
            if n3 > 0:
                S3c = work.tile([T, ncol], f32, tag="EMSc")
                nc.vector.tensor_mul(S3c[:, 0:n3], g_ps[:, 0:n3], M1c[:, BC:BC + n3])
                nc.vector.reduce_sum(red[:],
                                     S3c[:, 0:n3].rearrange("p (t b) -> p b t", b=BC),
                                     axis=mybir.AxisListType.X)
                nc.vector.tensor_add(pile[:], pile[:], red[:])
            # start / end parts
            if ci == 0:
                nc.vector.tensor_scalar(red[:], M1c[:, 0:BC], startT[:], None,
                                        op0=ALU.mult)
                nc.vector.tensor_add(pile[:], pile[:], red[:])
            if n0 + nn == ntb:
                nc.vector.tensor_scalar(red[:], M1c[:, nn - BC:nn], endT[:], None,
                                        op0=ALU.mult)
                nc.vector.tensor_add(pile[:], pile[:], red[:])
        sc_ps = psum_s.tile([1, BC], f32, tag="gps")
        nc.tensor.matmul(sc_ps[:], ones20f[:], pile[:], start=True, stop=True)
        score = state.tile([1, BC], f32, tag="score")
        nc.vector.tensor_copy(score[:], sc_ps[:])
        if "score" in dbg:
            nc.sync.dma_start(dbg["score"][:], score[:])
            nc.sync.dma_start(dbg["expem"][:], expem[:])

        # ---------- P5: exp-space forward scan ----------
        Scur = state.tile([T, BC], f32, tag="S0")
        nc.vector.tensor_scalar(Scur[:], expem[:, 0:BC], startexp[:], None,
                                op0=ALU.mult)
        for t in range(1, Lsteps):
            sp = psum_s.tile([T, BC], f32, tag="gps")
            nc.tensor.matmul(sp[:], mexp[:], Scur[:], start=True, stop=True)
            Snew = work.tile([T, BC], f32, tag=f"Sflip{t % 2}")
            nc.vector.tensor_mul(Snew[:], sp[:], expem[:, t * BC:(t + 1) * BC])
            Scur = Snew
        if "score" in dbg:
            nc.sync.dma_start(dbg["SL"][:], Scur[:])
        EE = state.tile([T, BC], f32, tag="EE")
        nc.vector.tensor_scalar(EE[:], Scur[:], endexp[:], None, op0=ALU.mult)
        z_ps = psum_s.tile([1, BC], f32, tag="gps")
        nc.tensor.matmul(z_ps[:], ones20f[:], EE[:], start=True, stop=True)
        vt = state.tile([1, BC], f32, tag="vt")
        nc.scalar.activation(vt[:], z_ps[:], AF.Ln)
        nc.vector.tensor_sub(vt[:], vt[:], score[:])
        nc.sync.dma_start(v_o[:], vt[:])
    nc.compile()
    return nc


def _host_inputs_from_dict(np_in, Lsteps):
    f = lambda k: np.asarray(np_in[k], np.float32)
    return _build_host_inputs(
        np.asarray(np_in["x"]), np.asarray(np_in["tags"]), f("emb"),
        f("w_ih0"), f("w_hh0"), f("b_ih0"), f("b_hh0"),
        f("w_ih1"), f("w_hh1"), f("b_ih1"), f("b_hh1"),
        f("fc_W"), f("fc_b"), f("crf_trans"), f("crf_start"), f("crf_end"),
        Lsteps)


TRACE = False          # set by test harnesses to capture an NTFF profile
LAST_RESULTS = None


def kernel(**inputs):
    global LAST_RESULTS
    np_in = {k: np.asarray(v) for k, v in inputs.items()}
    shared, per_core = _host_inputs_from_dict(np_in, L)
    nc = build_nc(L)
    in_maps = [dict(shared, **pc) for pc in per_core]
    LAST_RESULTS = run_bass_kernel_spmd(nc, in_maps, list(range(NC_CORES)),
                                        trace=TRACE)
    v = np.concatenate([r["v"][0] for r in LAST_RESULTS.results])
    return np.float32(np.mean(v))
